# revision 24
# baseline (speedup 1.0000x reference)
"""Causal self-attention (B=4, T=2048, D=1024, H=16) on 8 TRN2 NeuronCores.

Sharding: core c = (batch b = c//2, head-group hg = c%2). Each core computes
QKV projection for its 8 heads, causal flash-attention, and a row-parallel
partial out-projection; host sums the two partials per batch.

Per-core layouts (fp32 data; matmul operands typed float32r = TF32-class,
full PE rate; ~2e-4 relative error vs the fp32 reference):
  - S^T tiles [tk, tq] so exp+softmax-denominator and P@V both contract on
    partitions; denominator rides the PV matmul via a ones-column on V (M=65).
  - Normalization 1/rowsum broadcast across partitions via a step-0 DMA.
"""
import os
import numpy as np
import concourse.bacc as bacc
import concourse.mybir as mybir
from concourse.tile import TileContext
from concourse.bass_utils import run_bass_kernel_spmd

F32 = mybir.dt.float32
F32R = mybir.dt.float32r
AF = mybir.ActivationFunctionType

D, T, H, DH = 1024, 2048, 16, 64
LH = 8            # heads per core
TB = 512          # tq block
CH = 128          # tk chunk
NTB = T // TB     # 4
NCH = T // CH     # 16
KC = D // CH      # 8 contraction chunks
NPAIR = 4         # head pairs per core

PSA_BUFS = int(os.environ.get("K_PSA", "2"))
PSS_BUFS = int(os.environ.get("K_PSS", "2"))
PSO_BUFS = int(os.environ.get("K_PSO", "2"))
PT_BUFS = int(os.environ.get("K_PT", "6"))
XT_BUFS = int(os.environ.get("K_XT", "1"))
WQK_BUFS = int(os.environ.get("K_WQK", "4"))
MERGE_S = os.environ.get("K_MERGE_S", "1") == "1"
MASK_ENG = os.environ.get("K_MASK_ENG", "dve")
NORM_DMA = os.environ.get("K_NORM_DMA", "sync")





def build():
    nc = bacc.Bacc("TRN2")
    xt = nc.dram_tensor("xt", [D, T], F32R, kind="ExternalInput")
    w_qk = nc.dram_tensor("w_qk", [D, 1024], F32R, kind="ExternalInput")
    w_v = nc.dram_tensor("w_v", [D, 512], F32R, kind="ExternalInput")
    w_out = nc.dram_tensor("w_out", [512, D], F32R, kind="ExternalInput")
    b_qk = nc.dram_tensor("b_qk", [128, 8], F32, kind="ExternalInput")
    b_v = nc.dram_tensor("b_v", [128, 512], F32, kind="ExternalInput")
    b_out = nc.dram_tensor("b_out", [128, 8], F32, kind="ExternalInput")
    trimask = nc.dram_tensor("trimask", [128, 128], F32, kind="ExternalInput")
    outT = nc.dram_tensor("outT", [D, T], F32, kind="ExternalOutput")

    with TileContext(nc) as tc:
        with tc.tile_pool(name="const", bufs=1) as const, \
             tc.tile_pool(name="big", bufs=1) as big, \
             tc.tile_pool(name="qt", bufs=2) as qtp, \
             tc.tile_pool(name="xtp", bufs=XT_BUFS) as xtp, \
             tc.tile_pool(name="wqk", bufs=WQK_BUFS) as wqkp, \
             tc.tile_pool(name="yt", bufs=2) as ytp, \
             tc.tile_pool(name="pt", bufs=PT_BUFS) as ptp, \
             tc.tile_pool(name="small", bufs=3) as smallp, \
             tc.tile_pool(name="recp", bufs=1) as recp, \
             tc.tile_pool(name="ost", bufs=2) as ostp, \
             tc.tile_pool(name="psA", bufs=PSA_BUFS, space="PSUM") as psA, \
             tc.tile_pool(name="psS", bufs=PSS_BUFS, space="PSUM") as psS, \
             tc.tile_pool(name="psO", bufs=PSO_BUFS, space="PSUM") as psO:

            w_v_t = const.tile([128, KC, 512], F32R)
            w_out_t = const.tile([128, 4, 1024], F32R)
            b_qk_t = const.tile([128, 8], F32)
            nc.sync.dma_start(b_qk_t, b_qk[:])
            b_v_t = const.tile([128, 512], F32)
            nc.sync.dma_start(b_v_t, b_v[:])
            b_out_t = const.tile([128, 8], F32)
            nc.sync.dma_start(b_out_t, b_out[:])
            tri_t = const.tile([128, 128], F32)
            nc.sync.dma_start(tri_t, trimask[:])

            kT_t = big.tile([128, NPAIR, T], F32R)
            vaug_t = big.tile([128, NCH, LH, 65], F32R)
            nc.vector.memset(vaug_t[:, :, :, 64].bitcast(F32), 1.0)

            xt_r = xt[:].rearrange("(kc p) t -> p kc t", p=128)
            w_qk_r = w_qk[:].rearrange("(kc p) m -> p kc m", p=128)

            def emit_outproj(yT_prev, tb_prev):
                # partial out projection (row-parallel): out^T = w_out^T @ y^T
                for m in range(8):
                    ps = psA.tile([128, TB], F32, tag="proj")
                    for fc in range(4):
                        nc.tensor.matmul(
                            ps,
                            (w_out_t[:, fc, m * 128:(m + 1) * 128]),
                            (yT_prev[:, fc, :]),
                            start=(fc == 0), stop=(fc == 3))
                    ost = ostp.tile([128, TB], F32, tag="ost")
                    nc.vector.tensor_add(
                        out=ost, in0=ps, in1=b_out_t[:, m:m + 1].to_broadcast([128, TB]))
                    nc.sync.dma_start(
                        outT[m * 128:(m + 1) * 128, tb_prev * TB:(tb_prev + 1) * TB], ost)

            prev_y = None
            for tb in range(NTB):
                xt_t = xtp.tile([128, KC, TB], F32R)
                for kc in range(KC):
                    nc.sync.dma_start(
                        xt_t[:, kc, :], xt_r[:, kc, tb * TB:(tb + 1) * TB])

                # --- Q/K projection: features on partitions
                qT_t = qtp.tile([128, NPAIR, TB], F32R)
                for m in range(8):
                    wqk_t = wqkp.tile([128, KC, 128], F32R, tag="wqk")
                    nc.sync.dma_start(wqk_t, w_qk_r[:, :, m * 128:(m + 1) * 128])
                    ps = psA.tile([128, TB], F32, tag="proj")
                    for kc in range(KC):
                        nc.tensor.matmul(
                            ps,
                            (wqk_t[:, kc, :]),
                            (xt_t[:, kc, :]),
                            start=(kc == 0), stop=(kc == KC - 1))
                    dst = qT_t[:, m, :] if m < 4 else kT_t[:, m - 4, tb * TB:(tb + 1) * TB]
                    nc.vector.tensor_add(
                        out=dst, in0=ps, in1=b_qk_t[:, m:m + 1].to_broadcast([128, TB]))

                if tb == 0:
                    # deferred const loads: keep startup DMA queues clear for
                    # the first projection's xt/wqk tiles
                    nc.sync.dma_start(
                        w_v_t, w_v[:].rearrange("(kc p) m -> p kc m", p=128))
                    nc.sync.dma_start(
                        w_out_t, w_out[:].rearrange("(fc p) m -> p fc m", p=128))

                # --- V projection: tokens on partitions, into ones-augmented V
                for tt in range(4):
                    tg = tb * 4 + tt
                    ps = psA.tile([128, TB], F32, tag="proj")
                    for kc in range(KC):
                        nc.tensor.matmul(
                            ps,
                            (xt_t[:, kc, tt * 128:(tt + 1) * 128]),
                            (w_v_t[:, kc, :]),
                            start=(kc == 0), stop=(kc == KC - 1))
                    nc.vector.tensor_add(
                        out=vaug_t[:, tg, :, 0:64],
                        in0=ps.rearrange("p (h d) -> p h d", d=64),
                        in1=b_v_t.rearrange("p (h d) -> p h d", d=64))

                # out-proj of the previous block, emitted here so the next
                # projection outranks it for psA slots while attention drains
                if prev_y is not None:
                    emit_outproj(*prev_y)

                # --- causal attention for this tq block, all head pairs
                yT_t = ytp.tile([128, NPAIR, TB], F32R)
                nchunks = 4 * tb + 4

                def emit_s(p, j, pss):
                    c0 = max(0, 128 * j - TB * tb)
                    for hh in range(2):
                        lo = 64 * hh
                        nc.tensor.matmul(
                            pss[1 + hh][:, c0:TB],
                            kT_t[lo:lo + 64, p, j * 128:(j + 1) * 128],
                            qT_t[lo:lo + 64, p, c0:TB],
                            start=True, stop=True)

                mask_mul = nc.gpsimd.tensor_mul if MASK_ENG == "gpsimd" else nc.vector.tensor_mul

                def alloc_s():
                    if MERGE_S:
                        t = psS.tile([128, 2, TB], F32, tag="s")
                        return (t, t[:, 0, :], t[:, 1, :])
                    return (None,
                            psS.tile([128, TB], F32, tag="s", name="pssA"),
                            psS.tile([128, TB], F32, tag="s", name="pssB"))

                for p in range(NPAIR):
                    poA = psO.tile([128, TB], F32, tag="o")
                    poB = psO.tile([128, TB], F32, tag="o", name="poB")
                    po = (poA, poB)
                    pss_cur = alloc_s()
                    emit_s(p, 0, pss_cur)
                    for j in range(nchunks):
                        c0 = max(0, 128 * j - TB * tb)
                        # keep PE a chunk ahead of the ACT exp
                        if j + 1 < nchunks:
                            pss_next = alloc_s()
                            emit_s(p, j + 1, pss_next)
                        pt = ptp.tile([128, 2, TB], F32R, tag="pt")
                        if MERGE_S:
                            nc.scalar.activation(
                                pt[:, :, c0:TB], pss_cur[0][:, :, c0:TB],
                                AF.Exp, scale=0.125)
                        else:
                            for hh in range(2):
                                nc.scalar.activation(
                                    pt[:, hh, c0:TB], pss_cur[1 + hh][:, c0:TB],
                                    AF.Exp, scale=0.125)
                        if j >= 4 * tb:
                            mask_mul(
                                out=pt[:, :, c0:c0 + 128],
                                in0=pt[:, :, c0:c0 + 128],
                                in1=tri_t[:, None, :].to_broadcast([128, 2, 128]))
                        for hh in range(2):
                            nc.tensor.matmul(
                                po[hh][0:65, c0:TB],
                                vaug_t[:, j, 2 * p + hh, :],
                                pt[:, hh, c0:TB],
                                start=(j == 0), stop=(j == nchunks - 1))
                        if j + 1 < nchunks:
                            pss_cur = pss_next
                    # normalize: rowsum sits on psum partition 64 (V ones col)
                    rec = recp.tile([128, 2, TB], F32, tag="rec")
                    for hh in range(2):
                        nc.vector.reciprocal(rec[64:65, hh, :], po[hh][64:65, :])
                    for hh in range(2):
                        rbc = smallp.tile([64, TB], F32, tag="rbc")
                        (nc.gpsimd if NORM_DMA == "gpsimd" else nc.sync).dma_start(
                            rbc[:, None, :],
                            rec[64:65, hh, None, :].to_broadcast([1, 64, TB]))
                        if hh == 0:
                            nc.vector.tensor_mul(
                                out=yT_t[0:64, p, :], in0=po[0][0:64, :], in1=rbc)
                        else:
                            ty = smallp.tile([64, TB], F32R, tag="ty")
                            nc.vector.tensor_mul(out=ty, in0=po[1][0:64, :], in1=rbc)
                            (nc.gpsimd if NORM_DMA == "gpsimd" else nc.sync).dma_start(yT_t[64:128, p, :], ty)

                prev_y = (yT_t, tb)
            emit_outproj(*prev_y)
    nc.finalize()
    return nc


_NC = None


def _get_nc():
    global _NC
    if _NC is None:
        _NC = build()
    return _NC


def _make_in_maps(x, w_qkv, b_qkv, w_out, b_out):
    x = np.asarray(x, np.float32)
    w_qkv = np.asarray(w_qkv, np.float32)
    b_qkv = np.asarray(b_qkv, np.float32)
    w_out = np.asarray(w_out, np.float32)
    b_out = np.asarray(b_out, np.float32)
    trimask = np.triu(np.ones((128, 128), np.float32))
    in_maps = []
    for c in range(8):
        b, hg = divmod(c, 2)
        qcols = slice(512 * hg, 512 * hg + 512)
        kcols = slice(1024 + 512 * hg, 1024 + 512 * hg + 512)
        vcols = slice(2048 + 512 * hg, 2048 + 512 * hg + 512)
        in_maps.append({
            "xt": np.ascontiguousarray(x[b].T),
            "w_qk": np.ascontiguousarray(
                np.concatenate([w_qkv[:, qcols], w_qkv[:, kcols]], 1)),
            "w_v": np.ascontiguousarray(w_qkv[:, vcols]),
            "w_out": np.ascontiguousarray(w_out[512 * hg:512 * hg + 512, :]),
            "b_qk": np.ascontiguousarray(
                np.concatenate([b_qkv[qcols], b_qkv[kcols]]).reshape(8, 128).T),
            "b_v": np.ascontiguousarray(np.tile(b_qkv[vcols], (128, 1))),
            "b_out": np.ascontiguousarray(
                (b_out if hg == 0 else np.zeros_like(b_out)).reshape(8, 128).T),
            "trimask": trimask,
        })
    return in_maps


def _combine(results):
    out = np.zeros((4, T, D), np.float32)
    for c in range(8):
        out[c // 2] += results[c]["outT"].T
    return out


def _run(inputs, trace=False):
    nc = _get_nc()
    in_maps = _make_in_maps(**inputs)
    res = run_bass_kernel_spmd(nc, in_maps, core_ids=list(range(8)), trace=trace)
    return _combine(res.results), res


def kernel(x, w_qkv, b_qkv, w_out, b_out):
    out, _ = _run(dict(x=x, w_qkv=w_qkv, b_qkv=b_qkv, w_out=w_out, b_out=b_out))
    return out

